# revision 44
# baseline (speedup 1.0000x reference)
"""Trainium2 Bass kernel for the CLIP text/image concat multi-head classifier.

Full (unsharded) inputs in, full outputs out. 312 heads sharded 39-per-core
across 8 NeuronCores (head parallel); outputs concatenated along the class
axis on the host. No collectives.

v2 design (vs. baseline):
  - Text-side dot products (z_text = sum_d W_text[n,h,d]*text[n,d], the lin
    text terms, text norms) are B-independent per-class constants; they are
    folded into per-row biases on the host. This removes 44% of the
    steady-state weight DMA (the text half of C*_W1) and all N=1 matmuls.
  - 128-row tiles (rows padded 12168->12288, 96 tiles): full PE partition
    utilization (baseline used 104-row tiles = 81%).
  - Hidden pass per tile: 4|6 fp16 matmuls (ap 256) -> one ACT relu+bias
    into a persistent fp16 r buffer -> one paired bn_stats per 2 tiles.
  - BN tail (even/odd stat merge, var, rsqrt) batched over all 96 tiles in
    ~8 DVE/ACT ops on [128,96] arrays.
  - Output projection transposed: out[b,n] += r_tile[128rows,128b].T @
    At[128rows,39] accumulated over tiles, where At = (W2*gamma masked by
    head) * rsqrt(var+eps). Mean correction via a 257th r column and a
    final ones-row f32 matmul into the same PSUM group. ap=39 per matmul
    instead of the baseline's 257.
  - Outputs come out [B, 39] per core; host concat on axis 1, no transpose.
"""

import os
import sys
from contextlib import ExitStack

for _p in ("/opt/trn_rl_repo", "/root/.axon_site/_ro/trn_rl_repo"):
    if os.path.isdir(_p) and _p not in sys.path:
        sys.path.insert(0, _p)

import numpy as np
import concourse.bass as bass
import concourse.tile as tile
from concourse import bacc, mybir
from concourse.bass_utils import run_bass_kernel_spmd

F32 = mybir.dt.float32
F16 = mybir.dt.float16
AF = mybir.ActivationFunctionType
ALU = mybir.AluOpType

B, N, DE, DV, H = 256, 312, 512, 768, 312
EPS = 1e-5
NC = 8
NH = N // NC              # 39 heads per core
ROWS = NH * H             # 12168 (head, hidden) rows per core
TR = 128                  # rows per tile
NT = (ROWS + TR - 1) // TR  # 96 tiles (rows padded to 12288)
RP = NT * TR              # 12288 padded rows
C1D = DE // 128           # 4 contraction chunks
C2D = DV // 128           # 6
RW = B + 1                # r tile width: 256 batch + 1 mean col
G = int(os.environ.get("KG", "8"))   # weight tiles per DMA group
NG = NT // G              # DMA groups per classifier
KPROJ = int(os.environ.get("KPROJ", "0"))  # 1: At-stationary projection
KMSK = int(os.environ.get("KMSK", "0"))    # 1: on-chip head mask, small wgc
CONCAT_AXIS = 1


class Ctx:
    pass


def _make_persistents(nc, tc, ctx):
    c = Ctx()
    const = ctx.enter_context(tc.tile_pool(name="const", bufs=1))
    c.sp = ctx.enter_context(tc.tile_pool(name="sp", bufs=3))

    def mk(name, shape, dt):
        return const.tile(shape, dt, tag=name, name=name)

    c.imgT = mk("imgT", [128, C1D, B], F16)
    c.ioutT = mk("ioutT", [128, C2D, B], F16)
    c.w1iT = mk("w1iT", [128, C1D, NH], F16)
    c.w2iT = mk("w2iT", [128, C2D, NH], F16)
    c.ttsT = mk("ttsT", [128, C1D, NH], F16)
    c.lb1r = mk("lb1r", [1, NH], F32)
    c.lb2r = mk("lb2r", [1, NH], F32)
    c.zb1 = mk("zb1", [128, NT], F32)
    c.zb2 = mk("zb2", [128, NT], F32)
    if KMSK:
        c.wgc1 = mk("wgc1", [128, NT], F32)
        c.wgc2 = mk("wgc2", [128, NT], F32)
        c.atv1 = mk("atv1", [128, NT], F32)
        c.atv2 = mk("atv2", [128, NT], F32)
        # constant one-hot head mask: mask[p, t, n] = 1 iff (128t+p)//H == n,
        # i.e. 0 <= 128t + p - H*n <= H-1 — two affine half-planes.
        c.mask = mk("mask", [128, NT, NH], F16)
        nc.vector.memset(c.mask[:], 1.0)
        nc.gpsimd.affine_select(c.mask[:], c.mask[:],
                                pattern=[[128, NT], [-H, NH]],
                                compare_op=ALU.is_ge, fill=0.0,
                                base=0, channel_multiplier=1)
        nc.gpsimd.affine_select(c.mask[:], c.mask[:],
                                pattern=[[-128, NT], [H, NH]],
                                compare_op=ALU.is_ge, fill=0.0,
                                base=H - 1, channel_multiplier=-1)
    else:
        c.wg1 = mk("wg1", [128, NT, NH], F16)
        c.wg2 = mk("wg2", [128, NT, NH], F16)

    c.ones_col = mk("ones_col", [128, 1], F16)
    nc.vector.memset(c.ones_col[:], 1.0)
    c.ones_rowf = mk("ones_rowf", [1, 128], F32)
    nc.vector.memset(c.ones_rowf[:], 1.0)
    c.eps_col = mk("eps_col", [128, 1], F32)
    nc.vector.memset(c.eps_col[:], EPS)
    c.zrow = mk("zrow", [1, 512], F32)
    nc.vector.memset(c.zrow[:], 0.0)

    # persistent SBUF scratch
    c.rall1 = mk("rall1", [128, NT, RW], F16)
    c.rall2 = mk("rall2", [128, NT, RW], F16)
    c.st1 = mk("st1", [128, NT, 6], F32)
    c.st2 = mk("st2", [128, NT, 6], F32)
    c.inv1 = mk("inv1", [128, NT], F32)
    c.inv2 = mk("inv2", [128, NT], F32)
    return c


def _load_persistent_dmas(nc, c, ins):
    # order matters: imgT/ioutT first so PE can start early
    names = ("imgT", "ioutT", "w1iT", "w2iT", "ttsT", "lb1r", "lb2r",
             "zb1", "zb2") + (("wgc1", "wgc2") if KMSK else ("wg1", "wg2"))
    for name in names:
        nc.sync.dma_start(getattr(c, name)[:], ins[name][:])


def _phase_lin_logits(nc, c, spp, outs):
    sp = c.sp
    # lin1 / lin2: out[b, n] = sum_d img[b,d] W[n,d] + lbias[n]
    for (imt, wT, nch, lbr, oname) in (
            (c.imgT, c.w1iT, C1D, c.lb1r, "lin1_o"),
            (c.ioutT, c.w2iT, C2D, c.lb2r, "lin2_o")):
        for bh in range(2):
            lp = spp.tile([128, NH + 1], F32, tag="lp", bufs=2)
            for ch in range(nch):
                nc.tensor.matmul(lp[:, 0:NH],
                                 imt[:, ch, bh * 128:(bh + 1) * 128],
                                 wT[:, ch, :], start=(ch == 0), stop=False)
            nc.tensor.matmul(lp[:, 0:NH], c.ones_rowf[:], lbr[:],
                             start=False, stop=True)
            lsb = sp.tile([128, NH], F32, tag="lsb")
            nc.scalar.copy(lsb[:], lp[:, 0:NH])
            nc.sync.dma_start(outs[oname][bh * 128:(bh + 1) * 128, :], lsb[:])

    # logits: G[b,n] = sum_d img[b,d] * (text[n,d]*es/||t_n||), then * 1/||img_b||
    sq = sp.tile([128, C1D, B], F16, tag="sq")
    nc.vector.tensor_mul(sq[:], c.imgT[:], c.imgT[:])
    for bh in range(2):
        lpn = spp.tile([128, NH + 1], F32, tag="lp", bufs=2)
        gp = lpn[:, 0:NH]
        n2 = lpn[:, NH:NH + 1]
        for ch in range(C1D):
            nc.tensor.matmul(gp, c.imgT[:, ch, bh * 128:(bh + 1) * 128],
                             c.ttsT[:, ch, :], start=(ch == 0),
                             stop=(ch == C1D - 1))
        for ch in range(C1D):
            nc.tensor.matmul(n2, sq[:, ch, bh * 128:(bh + 1) * 128],
                             c.ones_col[:], start=(ch == 0),
                             stop=(ch == C1D - 1))
        nr = sp.tile([128, 1], F32, tag="nr")
        nc.scalar.sqrt(nr[:], n2)
        inv_i = sp.tile([128, 1], F32, tag="invi")
        nc.vector.reciprocal(inv_i[:], nr[:])
        lg = sp.tile([128, NH], F32, tag="lsb")
        nc.scalar.activation(lg[:], gp[:], AF.Copy, scale=inv_i[:])
        nc.sync.dma_start(outs["lgt_o"][bh * 128:(bh + 1) * 128, :], lg[:])


def _phase_hidden(nc, c, pools, wm_in, nch, imt, zb, rall, st, ph):
    wmp, zp = pools
    nbuf = int(os.environ.get("KZB", "5"))
    wbuf = int(os.environ.get("KWB", "3"))
    for g in range(NG):
        wm = wmp.tile([128, G, nch, TR], F16, tag="wmg", bufs=wbuf)
        nc.sync.dma_start(wm[:], wm_in[g])
        for i in range(G):
            t = g * G + i
            zps = zp.tile([128, B], F32, tag="zps", bufs=nbuf)
            for ch in range(nch):
                nc.tensor.matmul(zps[:], wm[:, i, ch, :], imt[:, ch, :],
                                 start=(ch == 0), stop=(ch == nch - 1))
            if ph & 4:
                nc.scalar.activation(rall[:, t, 0:B], zps[:], AF.Relu,
                                     bias=zb[:, t:t + 1])
            if ph & 8:
                nc.vector.bn_stats(st[:, t, :], rall[:, t, 0:B])


def _phase_bn_tail(nc, c, st, rall, inv_all, mim_o):
    # merge even/odd stats (each over 128 of the 256 batch):
    #   mean = (me+mo)/2 ; 256*var = (M2e+M2o) + 64*(me-mo)^2
    sp = c.sp
    me, mo = st[:, :, 1], st[:, :, 4]
    M2e, M2o = st[:, :, 2], st[:, :, 5]
    msum = sp.tile([128, NT], F32, tag="msum")
    nc.vector.tensor_add(msum[:], me, mo)
    dd = sp.tile([128, NT], F32, tag="dd")
    nc.vector.tensor_sub(dd[:], me, mo)
    dd2 = sp.tile([128, NT], F32, tag="dd2")
    nc.vector.tensor_mul(dd2[:], dd[:], dd[:])
    m2s = sp.tile([128, NT], F32, tag="m2s")
    nc.vector.tensor_add(m2s[:], M2e, M2o)
    vv = sp.tile([128, NT], F32, tag="vv")
    nc.vector.scalar_tensor_tensor(vv[:], dd2[:], 64.0, m2s[:],
                                   ALU.mult, ALU.add)
    sv = sp.tile([128, NT], F32, tag="sv")
    nc.scalar.activation(sv[:], vv[:], AF.Sqrt, bias=c.eps_col[:],
                         scale=1.0 / 256.0)
    nc.vector.reciprocal(inv_all[:], sv[:])
    if KPROJ:
        # mean column into r (col B), halved sum
        nc.scalar.activation(rall[:, :, B], msum[:], AF.Copy, scale=0.5)
    else:
        # export mean*inv per row; the host folds it into the correction
        mim = sp.tile([128, NT], F32, tag="mim")
        nc.vector.scalar_tensor_tensor(mim[:], msum[:], 0.5, inv_all[:],
                                       ALU.mult, ALU.mult)
        nc.sync.dma_start(mim_o[:], mim[:])


def _mk_at(nc, c, app, wg, wgc_atv, inv_all, t):
    At = app.tile([128, NH], F16, tag="At", bufs=4)
    if KMSK:
        nc.vector.tensor_scalar_mul(At[:], c.mask[:, t, :],
                                    wgc_atv[:, t:t + 1])
    else:
        nc.vector.tensor_scalar_mul(At[:], wg[:, t, :], inv_all[:, t:t + 1])
    return At


def _phase_project_swapped(nc, c, app, ppp, rall, wg, wgc_atv, inv_all,
                           out_o):
    # At is the stationary operand (128x39), rall tiles stream (ap=257).
    # Output PSUM is [39 heads, 257]: col 256 = mean projection; the host
    # transposes and applies the +cst-mean correction.
    sp = c.sp
    ppa = ppp.tile([NH, RW], F32, tag="ppa")
    for t in range(NT):
        At = _mk_at(nc, c, app, wg, wgc_atv, inv_all, t)
        nc.tensor.matmul(ppa[:], At[:], rall[:, t, :],
                         start=(t == 0), stop=(t == NT - 1))
    csb = sp.tile([NH, RW], F32, tag="csb")
    nc.scalar.copy(csb[:], ppa[:])
    nc.sync.dma_start(out_o[:], csb[:])


def _phase_project(nc, c, app, ppp, rall, wg, wgc, atv, inv_all, out_o):
    if KMSK:
        nc.vector.tensor_mul(atv[:], wgc[:], inv_all[:])
    # pp0/pp1/ppm share one PSUM bank: a full-bank dummy matmul opens the
    # group (start=True zeroes the whole 2KB zero-region) and another closes
    # it; their full-bank APs also order them around the partial-bank
    # accumulates in the scheduler. The +cst-mean correction happens on the
    # host: out rows 0:B are the raw sums, row B is the mean projection.
    if KPROJ:
        return _phase_project_swapped(nc, c, app, ppp, rall, wg, atv,
                                      inv_all, out_o)
    sp = c.sp
    ppa = ppp.tile([128, 512], F32, tag="ppa")
    pp0 = ppa[:, 0:NH]
    pp1 = ppa[:, 128:128 + NH]
    nc.tensor.matmul(ppa[:, 0:168], c.ones_rowf[:], c.zrow[:, 0:168],
                     start=True, stop=False)
    for t in range(NT):
        At = _mk_at(nc, c, app, wg, atv, inv_all, t)
        nc.tensor.matmul(pp0, rall[:, t, 0:128], At[:],
                         start=False, stop=False)
        nc.tensor.matmul(pp1, rall[:, t, 128:256], At[:],
                         start=False, stop=False)
    nc.tensor.matmul(ppa[:, 0:168], c.ones_rowf[:], c.zrow[:, 0:168],
                     start=False, stop=True)
    csb = sp.tile([128, 2, NH], F32, tag="csb")
    nc.scalar.copy(csb[:, 0, :], pp0)
    nc.scalar.copy(csb[:, 1, :], pp1)
    nc.sync.dma_start(out_o[0:B].rearrange("(h p) n -> p h n", h=2), csb[:])


def _emit_body(nc, tc, c, pools, ins, outs):
    # phase bits: 1 lin/logits, 2 hidden matmuls, 4 relu, 8 bn_stats,
    # 16 bn tail, 32 projection. Full kernel = 63.
    PH = int(os.environ.get("KPH", "63"))
    spp, wmp, zp, app, ppp = pools
    _load_persistent_dmas(nc, c, ins)
    if PH & 1:
        _phase_lin_logits(nc, c, spp, outs)
    if PH & 2:
        _phase_hidden(nc, c, (wmp, zp), ins["wm1"], C1D, c.imgT, c.zb1,
                      c.rall1, c.st1, PH)
        if PH & 16:
            _phase_bn_tail(nc, c, c.st1, c.rall1, c.inv1, outs["mim1_o"])
        _phase_hidden(nc, c, (wmp, zp), ins["wm2"], C2D, c.ioutT, c.zb2,
                      c.rall2, c.st2, PH)
        if PH & 16:
            _phase_bn_tail(nc, c, c.st2, c.rall2, c.inv2, outs["mim2_o"])
        if PH & 32:
            if KMSK:
                args1 = (None, c.wgc1, c.atv1)
                args2 = (None, c.wgc2, c.atv2)
            else:
                args1 = (c.wg1, None, None)
                args2 = (c.wg2, None, None)
            _phase_project(nc, c, app, ppp, c.rall1, *args1, c.inv1,
                           outs["cls1_o"])
            _phase_project(nc, c, app, ppp, c.rall2, *args2, c.inv2,
                           outs["cls2_o"])


def _build(loop_k=1):
    nc = bacc.Bacc("TRN2", target_bir_lowering=False, debug=False,
                   num_devices=NC)
    mk = nc.dram_tensor

    def inp(name, shape, dt):
        return mk(name, shape, dt, kind="ExternalInput").ap()

    ins = {
        "imgT": inp("imgT", [128, C1D * B], F16),
        "ioutT": inp("ioutT", [128, C2D * B], F16),
        "w1iT": inp("w1iT", [128, C1D * NH], F16),
        "w2iT": inp("w2iT", [128, C2D * NH], F16),
        "ttsT": inp("ttsT", [128, C1D * NH], F16),
        "lb1r": inp("lb1r", [1, NH], F32),
        "lb2r": inp("lb2r", [1, NH], F32),
        "zb1": inp("zb1", [128, NT], F32),
        "zb2": inp("zb2", [128, NT], F32),
        **({"wgc1": inp("wgc1", [128, NT], F32),
            "wgc2": inp("wgc2", [128, NT], F32)} if KMSK else
           {"wg1": inp("wg1", [128, NT * NH], F16),
            "wg2": inp("wg2", [128, NT * NH], F16)}),
        "wm1": inp("wm1", [NG, 128, G * C1D * TR], F16),
        "wm2": inp("wm2", [NG, 128, G * C2D * TR], F16),
    }
    def out_shape(k):
        if k.startswith("mim"):
            return [128, NT]
        if not k.startswith("cls"):
            return [B, NH]
        return [NH, B + 1] if KPROJ else [B, NH]

    outs = {
        k: mk(k, out_shape(k), F32, kind="ExternalOutput").ap()
        for k in ("lin1_o", "lin2_o", "cls1_o", "cls2_o", "lgt_o",
                  "mim1_o", "mim2_o")
    }

    unroll = int(os.environ.get("KUNROLL", "0"))
    with tile.TileContext(nc) as tc:
        with ExitStack() as ctx:
            c = _make_persistents(nc, tc, ctx)
            pools = (
                ctx.enter_context(tc.tile_pool(name="spp", bufs=2, space="PSUM")),
                ctx.enter_context(tc.tile_pool(name="wmp", bufs=3)),
                ctx.enter_context(tc.tile_pool(name="zp", bufs=5, space="PSUM")),
                ctx.enter_context(tc.tile_pool(name="app", bufs=8)),
                ctx.enter_context(tc.tile_pool(name="ppp", bufs=1, space="PSUM")),
            )
            if unroll > 1:
                for _ in range(unroll):
                    _emit_body(nc, tc, c, pools, ins, outs)
            elif loop_k > 1:
                with tc.For_i(0, loop_k, 1):
                    _emit_body(nc, tc, c, pools, ins, outs)
            else:
                _emit_body(nc, tc, c, pools, ins, outs)
    nc.compile()
    return nc


def _pack_T(x, nch, dtype=np.float16):
    # x: [rows, d] -> [128, nch*rows]; el [p, ch*rows + r] = x[r, ch*128+p]
    rows = x.shape[0]
    return np.ascontiguousarray(
        x.T.reshape(nch, 128, rows).transpose(1, 0, 2).reshape(128, nch * rows)
    ).astype(dtype)


def _pack_wm(w, nch):
    # w: [ROWS, nch*128] -> [NG, 128, G*nch*TR];
    # el [g, p, (i*nch+ch)*TR+r] = w[TR*(G*g+i)+r, 128*ch+p]
    wp = np.zeros((RP, nch * 128), np.float32)
    wp[:ROWS] = w
    return np.ascontiguousarray(
        wp.reshape(NG, G, TR, nch, 128).transpose(0, 4, 1, 3, 2)
        .reshape(NG, 128, G * nch * TR)
    ).astype(np.float16)


def _pack_cols(v):
    # v: [ROWS] -> [128, NT]; col t = v[t*TR:(t+1)*TR] (padded)
    vp = np.zeros((RP,), np.float32)
    vp[:ROWS] = v
    return np.ascontiguousarray(vp.reshape(NT, TR).T)


def _pack_wg(w2g):
    # w2g: [ROWS] -> [128, NT*NH] masked by head: el [p, t*NH+h] = w2g[t*128+p]
    # if (t*128+p)//H == h else 0
    arr = np.zeros((RP, NH), np.float32)
    r = np.arange(ROWS)
    arr[r, r // H] = w2g
    return np.ascontiguousarray(
        arr.reshape(NT, TR, NH).transpose(1, 0, 2).reshape(128, NT * NH)
    ).astype(np.float16)


def host_prep(inputs):
    f32 = np.float32
    g = {k: np.asarray(v, f32) for k, v in inputs.items()}
    image_embed, text_embed = g["image_embed"], g["text_embed"]
    image_out, text_out = g["image_out"], g["text_out"]

    imgT = _pack_T(image_embed, C1D)
    ioutT = _pack_T(image_out, C2D)
    es = np.exp(g["logit_scale"].astype(np.float64)).astype(f32)

    in_maps = []
    for cc in range(NC):
        S = slice(cc * NH, (cc + 1) * NH)
        # B-independent per-row hidden bias: text part of C*_W1 dotted with
        # the per-head text vector, plus C*_b1
        zt1 = np.einsum("nhd,nd->nh", g["C1_W1"][S][:, :, DE:], text_embed[S],
                        optimize=True) + g["C1_b1"][S]
        zt2 = np.einsum("nhd,nd->nh", g["C2_W1"][S][:, :, DV:], text_out[S],
                        optimize=True) + g["C2_b1"][S]

        w2g1 = (g["C1_W2"][S] * g["C1_gamma"][S]).reshape(ROWS)
        w2g2 = (g["C2_W2"][S] * g["C2_gamma"][S]).reshape(ROWS)
        cst1 = g["C1_b2"][S] + (g["C1_W2"][S] * g["C1_beta"][S]).sum(1)
        cst2 = g["C2_b2"][S] + (g["C2_W2"][S] * g["C2_beta"][S]).sum(1)
        lb1 = g["b1"][S] + (text_embed[S] * g["W1"][S, DE:]).sum(1)
        lb2 = g["b2"][S] + (text_out[S] * g["W2"][S, DV:]).sum(1)
        tsc = es / np.linalg.norm(text_embed[S], axis=1)

        in_maps.append({
            "imgT": imgT, "ioutT": ioutT,
            "w1iT": _pack_T(g["W1"][S, :DE], C1D),
            "w2iT": _pack_T(g["W2"][S, :DV], C2D),
            "ttsT": _pack_T(text_embed[S] * tsc[:, None], C1D),
            "lb1r": np.ascontiguousarray(lb1[None, :]),
            "lb2r": np.ascontiguousarray(lb2[None, :]),
            "_cst1": cst1, "_cst2": cst2,
            "_w2g1": w2g1, "_w2g2": w2g2,
            "zb1": _pack_cols(zt1.reshape(ROWS)),
            "zb2": _pack_cols(zt2.reshape(ROWS)),
            **({"wgc1": _pack_cols(w2g1), "wgc2": _pack_cols(w2g2)}
               if KMSK else
               {"wg1": _pack_wg(w2g1), "wg2": _pack_wg(w2g2)}),
            "wm1": _pack_wm(g["C1_W1"][S][:, :, :DE].reshape(ROWS, DE), C1D),
            "wm2": _pack_wm(g["C2_W1"][S][:, :, :DV].reshape(ROWS, DV), C2D),
        })
    return in_maps


_cache = {}


def _get_nc(loop_k=1):
    if loop_k not in _cache:
        _cache[loop_k] = _build(loop_k)
    return _cache[loop_k]


def _head_sums(w2g, mim):
    # mim: [128, NT] with mim[p, t] = row t*128+p; per-head sum of w2g*mim
    rv = np.zeros((RP,), np.float64)
    rv[:ROWS] = np.asarray(w2g, np.float64)
    rv *= np.asarray(mim, np.float64).T.reshape(RP)
    return rv[:ROWS].reshape(NH, H).sum(1).astype(np.float32)


def run(inputs, loop_k=1):
    nc = _get_nc(loop_k)
    in_maps = host_prep(inputs)
    aux = [(m.pop("_cst1"), m.pop("_cst2"), m.pop("_w2g1"), m.pop("_w2g2"))
           for m in in_maps]
    res = run_bass_kernel_spmd(nc, in_maps, core_ids=list(range(NC)))

    def get(cc, nm):
        r = res.results[cc][nm]
        if nm == "cls1_o" or nm == "cls2_o":
            i = 0 if nm == "cls1_o" else 1
            cst = aux[cc][i]
            if KPROJ:
                r = r.T
                return r[:B] + (cst - r[B])[None, :]
            ppm = _head_sums(aux[cc][2 + i], res.results[cc][f"mim{i+1}_o"])
            return r + (cst - ppm)[None, :]
        return r

    names = ("lin1_o", "lin2_o", "cls1_o", "cls2_o", "lgt_o")
    return tuple(
        np.ascontiguousarray(
            np.concatenate([get(c, nm) for c in range(NC)], axis=1))
        for nm in names
    )


def kernel(**inputs):
    return run(inputs, loop_k=1)



# revision 48
# speedup vs baseline: 1.0022x; 1.0022x over previous
"""Trainium2 Bass kernel for the CLIP text/image concat multi-head classifier.

Full (unsharded) inputs in, full outputs out. 312 heads sharded 39-per-core
across 8 NeuronCores (head parallel); outputs concatenated along the class
axis on the host. No collectives.

v5 design (vs. the v2 baseline at 234us/iter measured):
  - Text-side dot products are folded into per-row biases on the host (v2).
  - Weight streaming is grouped: 8 row-tiles per dma_start (KG), host-packed
    so each group is one contiguous-per-partition 1-1.5MB transfer. This
    cuts the per-DMA HWDGE queue overhead (~625ns each) from 192 weight
    DMAs/iter to 24, the dominant v2 bottleneck.
  - Hidden pass per tile: 4|6 fp16 matmuls (ap 256) -> ACT relu+bias ->
    fp16 r buffer -> bn_stats; triple-buffered weight groups (KWB) and 5
    PSUM accumulators (KZB) keep PE ~96% busy in steady state.
  - Projection: out[b,n] += r_tile[128rows,128b].T @ At[128rows,39], all
    three accumulators (b-halves) packed into ONE PSUM bank, with full-bank
    dummy matmuls opening/closing the accumulation group (PSUM start=True
    zeroes a whole 2KB zero-region, so co-resident groups are illegal).
  - At = mask * (w2*gamma*rsqrt(var+eps)) built from an on-chip constant
    one-hot head mask (two gpsimd affine_selects at startup) and a small
    [128,96] per-row coefficient input: replaces the 1.9MB/iter masked-wg
    DMA of v2 (KMSK).
  - The BN mean correction (+cst - proj(mean)) is applied on the HOST from
    an exported [128,96] mean*inv tensor, removing the mean column, its
    ACT write, and 192 ap=39 PE matmuls.
  - Measured on HW (quiet window): 110us/iter for the pre-KMSK/pre-mim
    variant vs 234us baseline; TimelineSim steady state 111.4us/iter,
    PE-bound (PE busy ~110us: hidden 102us fp16 roofline at 2.4GHz).
"""

import os
import sys
from contextlib import ExitStack

for _p in ("/opt/trn_rl_repo", "/root/.axon_site/_ro/trn_rl_repo"):
    if os.path.isdir(_p) and _p not in sys.path:
        sys.path.insert(0, _p)

import numpy as np
import concourse.bass as bass
import concourse.tile as tile
from concourse import bacc, mybir
from concourse.bass_utils import run_bass_kernel_spmd

F32 = mybir.dt.float32
F16 = mybir.dt.float16
AF = mybir.ActivationFunctionType
ALU = mybir.AluOpType

B, N, DE, DV, H = 256, 312, 512, 768, 312
EPS = 1e-5
NC = 8
NH = N // NC              # 39 heads per core
ROWS = NH * H             # 12168 (head, hidden) rows per core
TR = 128                  # rows per tile
NT = (ROWS + TR - 1) // TR  # 96 tiles (rows padded to 12288)
RP = NT * TR              # 12288 padded rows
C1D = DE // 128           # 4 contraction chunks
C2D = DV // 128           # 6
RW = B + 1                # r tile width: 256 batch + 1 mean col
G = int(os.environ.get("KG", "8"))   # weight tiles per DMA group
NG = NT // G              # DMA groups per classifier
KPROJ = int(os.environ.get("KPROJ", "0"))  # 1: At-stationary projection
KMSK = int(os.environ.get("KMSK", "1"))    # 1: on-chip head mask, small wgc
CONCAT_AXIS = 1


class Ctx:
    pass


def _make_persistents(nc, tc, ctx):
    c = Ctx()
    const = ctx.enter_context(tc.tile_pool(name="const", bufs=1))
    c.sp = ctx.enter_context(tc.tile_pool(name="sp", bufs=3))

    def mk(name, shape, dt):
        return const.tile(shape, dt, tag=name, name=name)

    c.imgT = mk("imgT", [128, C1D, B], F16)
    c.ioutT = mk("ioutT", [128, C2D, B], F16)
    c.w1iT = mk("w1iT", [128, C1D, NH], F16)
    c.w2iT = mk("w2iT", [128, C2D, NH], F16)
    c.ttsT = mk("ttsT", [128, C1D, NH], F16)
    c.lb1r = mk("lb1r", [1, NH], F32)
    c.lb2r = mk("lb2r", [1, NH], F32)
    c.zb1 = mk("zb1", [128, NT], F32)
    c.zb2 = mk("zb2", [128, NT], F32)
    if KMSK:
        c.wgc1 = mk("wgc1", [128, NT], F32)
        c.wgc2 = mk("wgc2", [128, NT], F32)
        c.atv1 = mk("atv1", [128, NT], F32)
        c.atv2 = mk("atv2", [128, NT], F32)
        # constant one-hot head mask: mask[p, t, n] = 1 iff (128t+p)//H == n,
        # i.e. 0 <= 128t + p - H*n <= H-1 — two affine half-planes.
        c.mask = mk("mask", [128, NT, NH], F16)
        nc.vector.memset(c.mask[:], 1.0)
        nc.gpsimd.affine_select(c.mask[:], c.mask[:],
                                pattern=[[128, NT], [-H, NH]],
                                compare_op=ALU.is_ge, fill=0.0,
                                base=0, channel_multiplier=1)
        nc.gpsimd.affine_select(c.mask[:], c.mask[:],
                                pattern=[[-128, NT], [H, NH]],
                                compare_op=ALU.is_ge, fill=0.0,
                                base=H - 1, channel_multiplier=-1)
    else:
        c.wg1 = mk("wg1", [128, NT, NH], F16)
        c.wg2 = mk("wg2", [128, NT, NH], F16)

    c.ones_col = mk("ones_col", [128, 1], F16)
    nc.vector.memset(c.ones_col[:], 1.0)
    c.ones_rowf = mk("ones_rowf", [1, 128], F32)
    nc.vector.memset(c.ones_rowf[:], 1.0)
    c.eps_col = mk("eps_col", [128, 1], F32)
    nc.vector.memset(c.eps_col[:], EPS)
    c.zrow = mk("zrow", [1, 512], F32)
    nc.vector.memset(c.zrow[:], 0.0)

    # persistent SBUF scratch
    c.rall1 = mk("rall1", [128, NT, RW], F16)
    c.rall2 = mk("rall2", [128, NT, RW], F16)
    c.st1 = mk("st1", [128, NT, 6], F32)
    c.st2 = mk("st2", [128, NT, 6], F32)
    c.inv1 = mk("inv1", [128, NT], F32)
    c.inv2 = mk("inv2", [128, NT], F32)
    return c


def _load_persistent_dmas(nc, c, ins):
    # order matters: imgT/ioutT first so PE can start early
    names = ("imgT", "ioutT", "w1iT", "w2iT", "ttsT", "lb1r", "lb2r",
             "zb1", "zb2") + (("wgc1", "wgc2") if KMSK else ("wg1", "wg2"))
    for name in names:
        nc.sync.dma_start(getattr(c, name)[:], ins[name][:])


def _phase_lin_logits(nc, c, spp, outs):
    sp = c.sp
    # lin1 / lin2: out[b, n] = sum_d img[b,d] W[n,d] + lbias[n]
    for (imt, wT, nch, lbr, oname) in (
            (c.imgT, c.w1iT, C1D, c.lb1r, "lin1_o"),
            (c.ioutT, c.w2iT, C2D, c.lb2r, "lin2_o")):
        for bh in range(2):
            lp = spp.tile([128, NH + 1], F32, tag="lp",
                          bufs=int(os.environ.get("KLB", "2")))
            for ch in range(nch):
                nc.tensor.matmul(lp[:, 0:NH],
                                 imt[:, ch, bh * 128:(bh + 1) * 128],
                                 wT[:, ch, :], start=(ch == 0), stop=False)
            nc.tensor.matmul(lp[:, 0:NH], c.ones_rowf[:], lbr[:],
                             start=False, stop=True)
            lsb = sp.tile([128, NH], F32, tag="lsb")
            nc.scalar.copy(lsb[:], lp[:, 0:NH])
            nc.sync.dma_start(outs[oname][bh * 128:(bh + 1) * 128, :], lsb[:])

    # logits: G[b,n] = sum_d img[b,d] * (text[n,d]*es/||t_n||), then * 1/||img_b||
    sq = sp.tile([128, C1D, B], F16, tag="sq")
    nc.vector.tensor_mul(sq[:], c.imgT[:], c.imgT[:])
    for bh in range(2):
        lpn = spp.tile([128, NH + 1], F32, tag="lp",
                       bufs=int(os.environ.get("KLB", "2")))
        gp = lpn[:, 0:NH]
        n2 = lpn[:, NH:NH + 1]
        for ch in range(C1D):
            nc.tensor.matmul(gp, c.imgT[:, ch, bh * 128:(bh + 1) * 128],
                             c.ttsT[:, ch, :], start=(ch == 0),
                             stop=(ch == C1D - 1))
        for ch in range(C1D):
            nc.tensor.matmul(n2, sq[:, ch, bh * 128:(bh + 1) * 128],
                             c.ones_col[:], start=(ch == 0),
                             stop=(ch == C1D - 1))
        nr = sp.tile([128, 1], F32, tag="nr")
        nc.scalar.sqrt(nr[:], n2)
        inv_i = sp.tile([128, 1], F32, tag="invi")
        nc.vector.reciprocal(inv_i[:], nr[:])
        lg = sp.tile([128, NH], F32, tag="lsb")
        nc.scalar.activation(lg[:], gp[:], AF.Copy, scale=inv_i[:])
        nc.sync.dma_start(outs["lgt_o"][bh * 128:(bh + 1) * 128, :], lg[:])


def _phase_hidden(nc, c, pools, wm_in, nch, imt, zb, rall, st, ph):
    wmp, zp = pools
    nbuf = int(os.environ.get("KZB", "5"))
    wbuf = int(os.environ.get("KWB", "3"))
    for g in range(NG):
        wm = wmp.tile([128, G, nch, TR], F16, tag="wmg", bufs=wbuf)
        nc.sync.dma_start(wm[:], wm_in[g])
        for i in range(G):
            t = g * G + i
            zps = zp.tile([128, B], F32, tag="zps", bufs=nbuf)
            for ch in range(nch):
                nc.tensor.matmul(zps[:], wm[:, i, ch, :], imt[:, ch, :],
                                 start=(ch == 0), stop=(ch == nch - 1))
            if ph & 4:
                nc.scalar.activation(rall[:, t, 0:B], zps[:], AF.Relu,
                                     bias=zb[:, t:t + 1])
            if ph & 8:
                nc.vector.bn_stats(st[:, t, :], rall[:, t, 0:B])


def _phase_bn_tail(nc, c, st, rall, inv_all, mim_o):
    # merge even/odd stats (each over 128 of the 256 batch):
    #   mean = (me+mo)/2 ; 256*var = (M2e+M2o) + 64*(me-mo)^2
    sp = c.sp
    me, mo = st[:, :, 1], st[:, :, 4]
    M2e, M2o = st[:, :, 2], st[:, :, 5]
    msum = sp.tile([128, NT], F32, tag="msum")
    nc.vector.tensor_add(msum[:], me, mo)
    dd = sp.tile([128, NT], F32, tag="dd")
    nc.vector.tensor_sub(dd[:], me, mo)
    dd2 = sp.tile([128, NT], F32, tag="dd2")
    nc.vector.tensor_mul(dd2[:], dd[:], dd[:])
    m2s = sp.tile([128, NT], F32, tag="m2s")
    nc.vector.tensor_add(m2s[:], M2e, M2o)
    vv = sp.tile([128, NT], F32, tag="vv")
    nc.vector.scalar_tensor_tensor(vv[:], dd2[:], 64.0, m2s[:],
                                   ALU.mult, ALU.add)
    sv = sp.tile([128, NT], F32, tag="sv")
    nc.scalar.activation(sv[:], vv[:], AF.Sqrt, bias=c.eps_col[:],
                         scale=1.0 / 256.0)
    nc.vector.reciprocal(inv_all[:], sv[:])
    if KPROJ:
        # mean column into r (col B), halved sum
        nc.scalar.activation(rall[:, :, B], msum[:], AF.Copy, scale=0.5)
    else:
        # export mean*inv per row; the host folds it into the correction
        mim = sp.tile([128, NT], F32, tag="mim")
        nc.vector.scalar_tensor_tensor(mim[:], msum[:], 0.5, inv_all[:],
                                       ALU.mult, ALU.mult)
        nc.sync.dma_start(mim_o[:], mim[:])


def _mk_at(nc, c, app, wg, wgc_atv, inv_all, t):
    At = app.tile([128, NH], F16, tag="At", bufs=4)
    if KMSK:
        nc.vector.tensor_scalar_mul(At[:], c.mask[:, t, :],
                                    wgc_atv[:, t:t + 1])
    else:
        nc.vector.tensor_scalar_mul(At[:], wg[:, t, :], inv_all[:, t:t + 1])
    return At


def _phase_project_swapped(nc, c, app, ppp, rall, wg, wgc_atv, inv_all,
                           out_o):
    # At is the stationary operand (128x39), rall tiles stream (ap=257).
    # Output PSUM is [39 heads, 257]: col 256 = mean projection; the host
    # transposes and applies the +cst-mean correction.
    sp = c.sp
    ppa = ppp.tile([NH, RW], F32, tag="ppa")
    for t in range(NT):
        At = _mk_at(nc, c, app, wg, wgc_atv, inv_all, t)
        nc.tensor.matmul(ppa[:], At[:], rall[:, t, :],
                         start=(t == 0), stop=(t == NT - 1))
    csb = sp.tile([NH, RW], F32, tag="csb")
    nc.scalar.copy(csb[:], ppa[:])
    nc.sync.dma_start(out_o[:], csb[:])


def _phase_project(nc, c, app, ppp, rall, wg, wgc, atv, inv_all, out_o):
    if KMSK:
        nc.vector.tensor_mul(atv[:], wgc[:], inv_all[:])
    # pp0/pp1/ppm share one PSUM bank: a full-bank dummy matmul opens the
    # group (start=True zeroes the whole 2KB zero-region) and another closes
    # it; their full-bank APs also order them around the partial-bank
    # accumulates in the scheduler. The +cst-mean correction happens on the
    # host: out rows 0:B are the raw sums, row B is the mean projection.
    if KPROJ:
        return _phase_project_swapped(nc, c, app, ppp, rall, wg, atv,
                                      inv_all, out_o)
    sp = c.sp
    ppa = ppp.tile([128, 512], F32, tag="ppa")
    pp0 = ppa[:, 0:NH]
    pp1 = ppa[:, 128:128 + NH]
    nc.tensor.matmul(ppa[:, 0:168], c.ones_rowf[:], c.zrow[:, 0:168],
                     start=True, stop=False)
    for t in range(NT):
        At = _mk_at(nc, c, app, wg, atv, inv_all, t)
        nc.tensor.matmul(pp0, rall[:, t, 0:128], At[:],
                         start=False, stop=False)
        nc.tensor.matmul(pp1, rall[:, t, 128:256], At[:],
                         start=False, stop=False)
    nc.tensor.matmul(ppa[:, 0:168], c.ones_rowf[:], c.zrow[:, 0:168],
                     start=False, stop=True)
    csb = sp.tile([128, 2, NH], F32, tag="csb")
    nc.scalar.copy(csb[:, 0, :], pp0)
    nc.scalar.copy(csb[:, 1, :], pp1)
    nc.sync.dma_start(out_o[0:B].rearrange("(h p) n -> p h n", h=2), csb[:])


def _emit_body(nc, tc, c, pools, ins, outs):
    # phase bits: 1 lin/logits, 2 hidden matmuls, 4 relu, 8 bn_stats,
    # 16 bn tail, 32 projection. Full kernel = 63.
    PH = int(os.environ.get("KPH", "63"))
    spp, wmp, zp, app, ppp = pools
    _load_persistent_dmas(nc, c, ins)
    if PH & 1:
        _phase_lin_logits(nc, c, spp, outs)
    if PH & 2:
        _phase_hidden(nc, c, (wmp, zp), ins["wm1"], C1D, c.imgT, c.zb1,
                      c.rall1, c.st1, PH)
        if PH & 16:
            _phase_bn_tail(nc, c, c.st1, c.rall1, c.inv1, outs["mim1_o"])
        _phase_hidden(nc, c, (wmp, zp), ins["wm2"], C2D, c.ioutT, c.zb2,
                      c.rall2, c.st2, PH)
        if PH & 16:
            _phase_bn_tail(nc, c, c.st2, c.rall2, c.inv2, outs["mim2_o"])
        if PH & 32:
            if KMSK:
                args1 = (None, c.wgc1, c.atv1)
                args2 = (None, c.wgc2, c.atv2)
            else:
                args1 = (c.wg1, None, None)
                args2 = (c.wg2, None, None)
            _phase_project(nc, c, app, ppp, c.rall1, *args1, c.inv1,
                           outs["cls1_o"])
            _phase_project(nc, c, app, ppp, c.rall2, *args2, c.inv2,
                           outs["cls2_o"])


def _build(loop_k=1):
    nc = bacc.Bacc("TRN2", target_bir_lowering=False, debug=False,
                   num_devices=NC)
    mk = nc.dram_tensor

    def inp(name, shape, dt):
        return mk(name, shape, dt, kind="ExternalInput").ap()

    ins = {
        "imgT": inp("imgT", [128, C1D * B], F16),
        "ioutT": inp("ioutT", [128, C2D * B], F16),
        "w1iT": inp("w1iT", [128, C1D * NH], F16),
        "w2iT": inp("w2iT", [128, C2D * NH], F16),
        "ttsT": inp("ttsT", [128, C1D * NH], F16),
        "lb1r": inp("lb1r", [1, NH], F32),
        "lb2r": inp("lb2r", [1, NH], F32),
        "zb1": inp("zb1", [128, NT], F32),
        "zb2": inp("zb2", [128, NT], F32),
        **({"wgc1": inp("wgc1", [128, NT], F32),
            "wgc2": inp("wgc2", [128, NT], F32)} if KMSK else
           {"wg1": inp("wg1", [128, NT * NH], F16),
            "wg2": inp("wg2", [128, NT * NH], F16)}),
        "wm1": inp("wm1", [NG, 128, G * C1D * TR], F16),
        "wm2": inp("wm2", [NG, 128, G * C2D * TR], F16),
    }
    def out_shape(k):
        if k.startswith("mim"):
            return [128, NT]
        if not k.startswith("cls"):
            return [B, NH]
        return [NH, B + 1] if KPROJ else [B, NH]

    outs = {
        k: mk(k, out_shape(k), F32, kind="ExternalOutput").ap()
        for k in ("lin1_o", "lin2_o", "cls1_o", "cls2_o", "lgt_o",
                  "mim1_o", "mim2_o")
    }

    unroll = int(os.environ.get("KUNROLL", "0"))
    with tile.TileContext(nc) as tc:
        with ExitStack() as ctx:
            c = _make_persistents(nc, tc, ctx)
            pools = (
                ctx.enter_context(tc.tile_pool(name="spp", bufs=2, space="PSUM")),
                ctx.enter_context(tc.tile_pool(name="wmp", bufs=3)),
                ctx.enter_context(tc.tile_pool(name="zp", bufs=5, space="PSUM")),
                ctx.enter_context(tc.tile_pool(name="app", bufs=8)),
                ctx.enter_context(tc.tile_pool(name="ppp", bufs=1, space="PSUM")),
            )
            if unroll > 1:
                for _ in range(unroll):
                    _emit_body(nc, tc, c, pools, ins, outs)
            elif loop_k > 1:
                with tc.For_i(0, loop_k, 1):
                    _emit_body(nc, tc, c, pools, ins, outs)
            else:
                _emit_body(nc, tc, c, pools, ins, outs)
    nc.compile()
    return nc


def _pack_T(x, nch, dtype=np.float16):
    # x: [rows, d] -> [128, nch*rows]; el [p, ch*rows + r] = x[r, ch*128+p]
    rows = x.shape[0]
    return np.ascontiguousarray(
        x.T.reshape(nch, 128, rows).transpose(1, 0, 2).reshape(128, nch * rows)
    ).astype(dtype)


def _pack_wm(w, nch):
    # w: [ROWS, nch*128] -> [NG, 128, G*nch*TR];
    # el [g, p, (i*nch+ch)*TR+r] = w[TR*(G*g+i)+r, 128*ch+p]
    wp = np.zeros((RP, nch * 128), np.float32)
    wp[:ROWS] = w
    return np.ascontiguousarray(
        wp.reshape(NG, G, TR, nch, 128).transpose(0, 4, 1, 3, 2)
        .reshape(NG, 128, G * nch * TR)
    ).astype(np.float16)


def _pack_cols(v):
    # v: [ROWS] -> [128, NT]; col t = v[t*TR:(t+1)*TR] (padded)
    vp = np.zeros((RP,), np.float32)
    vp[:ROWS] = v
    return np.ascontiguousarray(vp.reshape(NT, TR).T)


def _pack_wg(w2g):
    # w2g: [ROWS] -> [128, NT*NH] masked by head: el [p, t*NH+h] = w2g[t*128+p]
    # if (t*128+p)//H == h else 0
    arr = np.zeros((RP, NH), np.float32)
    r = np.arange(ROWS)
    arr[r, r // H] = w2g
    return np.ascontiguousarray(
        arr.reshape(NT, TR, NH).transpose(1, 0, 2).reshape(128, NT * NH)
    ).astype(np.float16)


def host_prep(inputs):
    f32 = np.float32
    g = {k: np.asarray(v, f32) for k, v in inputs.items()}
    image_embed, text_embed = g["image_embed"], g["text_embed"]
    image_out, text_out = g["image_out"], g["text_out"]

    imgT = _pack_T(image_embed, C1D)
    ioutT = _pack_T(image_out, C2D)
    es = np.exp(g["logit_scale"].astype(np.float64)).astype(f32)

    in_maps = []
    for cc in range(NC):
        S = slice(cc * NH, (cc + 1) * NH)
        # B-independent per-row hidden bias: text part of C*_W1 dotted with
        # the per-head text vector, plus C*_b1
        zt1 = np.einsum("nhd,nd->nh", g["C1_W1"][S][:, :, DE:], text_embed[S],
                        optimize=True) + g["C1_b1"][S]
        zt2 = np.einsum("nhd,nd->nh", g["C2_W1"][S][:, :, DV:], text_out[S],
                        optimize=True) + g["C2_b1"][S]

        w2g1 = (g["C1_W2"][S] * g["C1_gamma"][S]).reshape(ROWS)
        w2g2 = (g["C2_W2"][S] * g["C2_gamma"][S]).reshape(ROWS)
        cst1 = g["C1_b2"][S] + (g["C1_W2"][S] * g["C1_beta"][S]).sum(1)
        cst2 = g["C2_b2"][S] + (g["C2_W2"][S] * g["C2_beta"][S]).sum(1)
        lb1 = g["b1"][S] + (text_embed[S] * g["W1"][S, DE:]).sum(1)
        lb2 = g["b2"][S] + (text_out[S] * g["W2"][S, DV:]).sum(1)
        tsc = es / np.linalg.norm(text_embed[S], axis=1)

        in_maps.append({
            "imgT": imgT, "ioutT": ioutT,
            "w1iT": _pack_T(g["W1"][S, :DE], C1D),
            "w2iT": _pack_T(g["W2"][S, :DV], C2D),
            "ttsT": _pack_T(text_embed[S] * tsc[:, None], C1D),
            "lb1r": np.ascontiguousarray(lb1[None, :]),
            "lb2r": np.ascontiguousarray(lb2[None, :]),
            "_cst1": cst1, "_cst2": cst2,
            "_w2g1": w2g1, "_w2g2": w2g2,
            "zb1": _pack_cols(zt1.reshape(ROWS)),
            "zb2": _pack_cols(zt2.reshape(ROWS)),
            **({"wgc1": _pack_cols(w2g1), "wgc2": _pack_cols(w2g2)}
               if KMSK else
               {"wg1": _pack_wg(w2g1), "wg2": _pack_wg(w2g2)}),
            "wm1": _pack_wm(g["C1_W1"][S][:, :, :DE].reshape(ROWS, DE), C1D),
            "wm2": _pack_wm(g["C2_W1"][S][:, :, :DV].reshape(ROWS, DV), C2D),
        })
    return in_maps


_cache = {}


def _get_nc(loop_k=1):
    if loop_k not in _cache:
        _cache[loop_k] = _build(loop_k)
    return _cache[loop_k]


def _head_sums(w2g, mim):
    # mim: [128, NT] with mim[p, t] = row t*128+p; per-head sum of w2g*mim
    rv = np.zeros((RP,), np.float64)
    rv[:ROWS] = np.asarray(w2g, np.float64)
    rv *= np.asarray(mim, np.float64).T.reshape(RP)
    return rv[:ROWS].reshape(NH, H).sum(1).astype(np.float32)


def run(inputs, loop_k=1):
    nc = _get_nc(loop_k)
    in_maps = host_prep(inputs)
    aux = [(m.pop("_cst1"), m.pop("_cst2"), m.pop("_w2g1"), m.pop("_w2g2"))
           for m in in_maps]
    res = run_bass_kernel_spmd(nc, in_maps, core_ids=list(range(NC)))

    def get(cc, nm):
        r = res.results[cc][nm]
        if nm == "cls1_o" or nm == "cls2_o":
            i = 0 if nm == "cls1_o" else 1
            cst = aux[cc][i]
            if KPROJ:
                r = r.T
                return r[:B] + (cst - r[B])[None, :]
            ppm = _head_sums(aux[cc][2 + i], res.results[cc][f"mim{i+1}_o"])
            return r + (cst - ppm)[None, :]
        return r

    names = ("lin1_o", "lin2_o", "cls1_o", "cls2_o", "lgt_o")
    return tuple(
        np.ascontiguousarray(
            np.concatenate([get(c, nm) for c in range(NC)], axis=1))
        for nm in names
    )


def kernel(**inputs):
    return run(inputs, loop_k=1)

